# revision 8
# baseline (speedup 1.0000x reference)
"""BitLinear (ternary-weight linear + global activation requant) on 8 TRN2 cores.

Computation (see reference):
    wq  = ternarize(weight * scale, thr = 0.7*mean|weight*scale|)   # {-1,0,+1}
    out = x @ wq.T + bias
    s   = 255 / (max(out) - min(out));  out = round(out*s)/s

Sharding: 2x4 grid over (tokens, out_features).  Each core computes a
[4096 tok, 1024 out] shard contracting over the full K=4096.

v2 design (vs the staged baseline):
  - fp16 x / fp16 wq matmuls (same PE rate as bf16, much lower quant error)
  - pass-1 |W| scan split across the 2 row-replica cores (half the DMA)
  - ternarize via clamp(rne(w*beta), -1, 1) in 3 cheap DVE/ScalarE ops,
    streamed under the start of the MM phase (slice-level wq deps)
  - raw y kept resident in SBUF as f16 -- no DRAM staging round trip
  - first 4 blocks interleaved ko-major so the PE rides the pass-2 W DMA
    stream; remaining blocks sequential with pipelined PSUM banks
  - tiny collectives as AllGather (4.6us floor) + local reduce
  - requant tail split across DVE and ScalarE, f16 output (host upcasts)
"""

import numpy as np
import ml_dtypes

import concourse.bass as bass
import concourse.mybir as mybir
import concourse.tile as tile
from concourse.tile import add_dep_helper
from concourse import bacc
from concourse import bass_utils

F32 = mybir.dt.float32
F16 = mybir.dt.float16

# Full problem shape
B, S, D_IN, D_OUT = 4, 2048, 4096, 4096
N_CORES = 8
GRID_R, GRID_C = 2, 4  # token shards x out-feature shards

TOK_PC = (B * S) // GRID_R      # 4096 tokens per core
OUT_PC = D_OUT // GRID_C        # 1024 out features per core
KO = D_IN // 128                # 32 contraction slabs
TOKB = 128                      # tokens per block
NB = TOK_PC // TOKB             # 32 blocks
OGS, OGW = 2, 512               # out groups per block (PSUM tiles)
NFLIGHT = 4                     # blocks in flight (NFLIGHT*OGS = 8 PSUM banks)

MAGIC = 12582912.0              # f32 RNE magic (1.5 * 2^23)


def build_kernel(debug: bool = False, repeat: int = 1):
    nc = bacc.Bacc(
        "TRN2",
        target_bir_lowering=False,
        debug=debug,
        enable_asserts=False,
        num_devices=N_CORES,
    )

    xt = nc.declare_dram_parameter("xt", [NB, 128, KO, TOKB], F16, isOutput=False)
    wt = nc.declare_dram_parameter("wt", [D_IN, OUT_PC], F32, isOutput=False)
    wt1 = nc.declare_dram_parameter("wt1", [D_IN // 2, OUT_PC], F32, isOutput=False)
    biasv = nc.declare_dram_parameter("biasv", [OUT_PC], F16, isOutput=False)
    scalev = nc.declare_dram_parameter("scalev", [1], F32, isOutput=False)
    out = nc.declare_dram_parameter("outv", [TOK_PC, OUT_PC], F16, isOutput=True)

    xt_ap = xt.ap()
    wt_ap = wt.ap()
    wt1_ap = wt1.ap()
    out_ap = out.ap()

    with tile.TileContext(nc) as tc:
        with (
            tc.tile_pool(name="const", bufs=1) as const_pool,
            tc.tile_pool(name="wpool", bufs=2) as wpool,
            tc.tile_pool(name="wq", bufs=1) as wq_pool,
            tc.tile_pool(name="xbuf", bufs=6) as x_pool,
            tc.tile_pool(name="ybuf", bufs=1) as y_pool,
            tc.tile_pool(name="psum", bufs=1, space="PSUM") as psum_pool,
            tc.tile_pool(name="dram", bufs=1, space="DRAM") as dram_pool,
        ):

            def newton_recip(out_ap2, in_ap, tag):
                # r_{i+1} = r_i*(2 - x*r_i), two Newton steps on InstReciprocal
                r0 = const_pool.tile([1, 1], F32, tag=f"{tag}_r0")
                nc.vector.reciprocal(r0, in_ap)
                e = const_pool.tile([1, 1], F32, tag=f"{tag}_e")
                for _ in range(2):
                    nc.vector.tensor_scalar(
                        e, in_ap, r0, None, mybir.AluOpType.mult
                    )
                    nc.vector.tensor_scalar(
                        e, e, -1.0, 2.0, mybir.AluOpType.mult, mybir.AluOpType.add
                    )
                    nc.vector.tensor_mul(r0, r0, e)
                nc.vector.tensor_copy(out=out_ap2, in_=r0)

            def body():
                # ---- consts ----
                scale_sb = const_pool.tile([1, 1], F32, tag="scale_sb")
                nc.sync.dma_start(scale_sb, scalev.ap()[None, :])
                bias_sb = const_pool.tile([1, OUT_PC], F16, tag="bias_sb")
                nc.sync.dma_start(bias_sb, biasv.ap()[None, :])
                bias_b = const_pool.tile([128, OUT_PC], F16, tag="bias_b")
                nc.gpsimd.partition_broadcast(bias_b, bias_sb)

                # ---- pass-1: |W| sum over this core's half of its W shard ----
                wsum = const_pool.tile([128, KO // 2], F32, tag="wsum")
                p1_last = None
                for k in range(KO // 2):
                    s1 = wpool.tile([128, OUT_PC], F32, tag="wslab", bufs=3)
                    p1_last = nc.sync.dma_start(
                        s1, wt1_ap[k * 128:(k + 1) * 128, :]
                    )
                    nc.vector.tensor_reduce(
                        wsum[:, k:k + 1], s1,
                        axis=mybir.AxisListType.X,
                        op=mybir.AluOpType.add, apply_absolute_value=True,
                    )
                wsum1 = const_pool.tile([128, 1], F32, tag="wsum1")
                nc.vector.tensor_reduce(
                    wsum1, wsum, axis=mybir.AxisListType.X, op=mybir.AluOpType.add
                )
                wsum_all = const_pool.tile([128, 1], F32, tag="wsum_all")
                nc.gpsimd.partition_all_reduce(
                    wsum_all, wsum1, 128, bass.bass_isa.ReduceOp.add
                )
                cc1_in = dram_pool.tile([1, 1], F32, tag="cc1_in")
                cc1_out = dram_pool.tile([N_CORES, 1], F32, tag="cc1_out")
                nc.sync.dma_start(cc1_in, wsum_all[0:1, :])
                nc.gpsimd.collective_compute(
                    "AllGather",
                    mybir.AluOpType.bypass,
                    replica_groups=[list(range(N_CORES))],
                    ins=[cc1_in.opt()],
                    outs=[cc1_out.opt()],
                )
                s8 = const_pool.tile([1, N_CORES], F32, tag="s8")
                nc.sync.dma_start(s8, cc1_out.rearrange("g c -> c g"))
                s_glob = const_pool.tile([1, 1], F32, tag="s_glob")
                nc.vector.tensor_reduce(
                    s_glob, s8, axis=mybir.AxisListType.X, op=mybir.AluOpType.add
                )

                # ---- beta = 0.5*scale / thr;  thr = 0.7/n * S_glob * |scale| ----
                absscale = const_pool.tile([1, 1], F32, tag="absscale")
                nc.vector.tensor_reduce(
                    absscale, scale_sb, axis=mybir.AxisListType.X,
                    op=mybir.AluOpType.max, apply_absolute_value=True,
                )
                r1 = const_pool.tile([1, 1], F32, tag="r1")
                newton_recip(r1, s_glob, "r1")
                r2 = const_pool.tile([1, 1], F32, tag="r2")
                newton_recip(r2, absscale, "r2")
                n_w = float(D_IN) * float(D_OUT)
                bco = float(np.float32(0.5 * n_w / 0.7))
                beta = const_pool.tile([1, 1], F32, tag="beta")
                nc.vector.tensor_mul(beta, r1, r2)
                nc.vector.tensor_mul(beta, beta, scale_sb)
                nc.vector.tensor_scalar_mul(beta, beta, bco)
                beta_b = const_pool.tile([128, 1], F32, tag="beta_b")
                nc.gpsimd.partition_broadcast(beta_b, beta)

                # ---- x prefetch for the first NFLIGHT blocks ----
                xts = {}
                prev = p1_last
                for b in range(NFLIGHT):
                    xtile = x_pool.tile([128, KO, TOKB], F16, tag="x_tile")
                    d = nc.sync.dma_start(xtile, xt_ap[b])
                    add_dep_helper(d.ins, prev.ins, sync=True,
                                   reason="x prefetch after pass-1 / chain")
                    prev = d
                    xts[b] = xtile

                # ---- pass-2 W stream + ternarize into wq (f16) ----
                wq = wq_pool.tile([128, KO, OUT_PC], F16, tag="wq")
                d2first = None
                for ko in range(KO):
                    s2 = wpool.tile([128, OUT_PC], F32, tag="wslab", bufs=3)
                    d2 = nc.sync.dma_start(s2, wt_ap[ko * 128:(ko + 1) * 128, :])
                    if d2first is None:
                        add_dep_helper(d2.ins, p1_last.ins, sync=True,
                                       reason="pass-2 W after pass-1 W DMA")
                        d2first = d2
                    a = wpool.tile([128, OUT_PC], F32, tag="wslab", bufs=3)
                    nc.vector.tensor_scalar(
                        a, s2, beta_b[:, 0:1], MAGIC,
                        mybir.AluOpType.mult, mybir.AluOpType.add,
                    )
                    bt = wpool.tile([128, OUT_PC], F16, tag="b16", bufs=2)
                    nc.scalar.activation(
                        bt, a, mybir.ActivationFunctionType.Copy,
                        bias=-MAGIC, scale=1.0,
                    )
                    nc.vector.tensor_scalar(
                        wq[:, ko, :], bt, 1.0, -1.0,
                        mybir.AluOpType.min, mybir.AluOpType.max,
                    )

                # ---- MM phase ----
                y = y_pool.tile([128, NB, OUT_PC], F16, tag="y")
                maxst = const_pool.tile([128, NB * OGS], F32, tag="maxst")
                minst = const_pool.tile([128, NB * OGS], F32, tag="minst")

                def psums_for(b):
                    return [
                        psum_pool.tile([128, OGW], F32, name=f"ps_{b % NFLIGHT}_{og}")
                        for og in range(OGS)
                    ]

                def drain(b, ps):
                    for og in range(OGS):
                        sl = slice(og * OGW, (og + 1) * OGW)
                        nc.vector.tensor_add(y[:, b, sl], ps[og], bias_b[:, sl])
                        idx = b * OGS + og
                        nc.vector.tensor_reduce(
                            maxst[:, idx:idx + 1], y[:, b, sl],
                            axis=mybir.AxisListType.X, op=mybir.AluOpType.max,
                        )
                        nc.vector.tensor_reduce(
                            minst[:, idx:idx + 1], y[:, b, sl],
                            axis=mybir.AxisListType.X, op=mybir.AluOpType.min,
                        )

                # phase A: first NFLIGHT blocks interleaved ko-major, pacing
                # the PE to the pass-2 DMA/ternarize stream
                psA = {b: psums_for(b) for b in range(NFLIGHT)}
                for ko in range(KO):
                    for b in range(NFLIGHT):
                        lhsT = xts[b][:, ko, :]
                        for og in range(OGS):
                            nc.tensor.matmul(
                                psA[b][og], lhsT,
                                wq[:, ko, og * OGW:(og + 1) * OGW],
                                start=(ko == 0), stop=(ko == KO - 1),
                            )
                for b in range(NFLIGHT):
                    drain(b, psA[b])

                # phase B: remaining blocks, sequential with pipelined banks
                for b in range(NFLIGHT, NB):
                    # prefetch x two blocks ahead
                    pf = b + 2 if b + 2 < NB else None
                    if b not in xts:
                        pass
                    xtile = xts.pop(b, None)
                    if xtile is None:
                        xtile = x_pool.tile([128, KO, TOKB], F16, tag="x_tile")
                        nc.sync.dma_start(xtile, xt_ap[b])
                    if pf is not None and pf not in xts:
                        xp = x_pool.tile([128, KO, TOKB], F16, tag="x_tile")
                        nc.sync.dma_start(xp, xt_ap[pf])
                        xts[pf] = xp
                    ps = psums_for(b)
                    for ko in range(KO):
                        lhsT = xtile[:, ko, :]
                        for og in range(OGS):
                            nc.tensor.matmul(
                                ps[og], lhsT,
                                wq[:, ko, og * OGW:(og + 1) * OGW],
                                start=(ko == 0), stop=(ko == KO - 1),
                            )
                    drain(b, ps)

                # ---- global max/min -> s ----
                lmax = const_pool.tile([128, 1], F32, tag="lmax")
                lmin = const_pool.tile([128, 1], F32, tag="lmin")
                nc.vector.tensor_reduce(
                    lmax, maxst, axis=mybir.AxisListType.X, op=mybir.AluOpType.max
                )
                nc.vector.tensor_reduce(
                    lmin, minst, axis=mybir.AxisListType.X, op=mybir.AluOpType.min
                )
                st2 = const_pool.tile([128, 2], F32, tag="st2")
                nc.vector.tensor_copy(out=st2[:, 0:1], in_=lmax)
                nc.vector.tensor_scalar_mul(st2[:, 1:2], lmin, -1.0)
                st2r = const_pool.tile([128, 2], F32, tag="st2r")
                nc.gpsimd.partition_all_reduce(
                    st2r, st2, 128, bass.bass_isa.ReduceOp.max
                )
                cc2_in = dram_pool.tile([1, 2], F32, tag="cc2_in")
                cc2_out = dram_pool.tile([N_CORES, 2], F32, tag="cc2_out")
                nc.sync.dma_start(cc2_in, st2r[0:1, :])
                nc.gpsimd.collective_compute(
                    "AllGather",
                    mybir.AluOpType.bypass,
                    replica_groups=[list(range(N_CORES))],
                    ins=[cc2_in.opt()],
                    outs=[cc2_out.opt()],
                )
                gst = const_pool.tile([1, 2, N_CORES], F32, tag="gst")
                nc.sync.dma_start(gst, cc2_out.rearrange("g c -> c g")[None])
                g2 = const_pool.tile([1, 2, 1], F32, tag="g2")
                nc.vector.tensor_reduce(
                    g2, gst, axis=mybir.AxisListType.X, op=mybir.AluOpType.max
                )
                rng = const_pool.tile([1, 1], F32, tag="rng")  # max - min
                nc.vector.tensor_reduce(
                    rng, g2[:, :, 0], axis=mybir.AxisListType.X,
                    op=mybir.AluOpType.add,
                )
                # sq = [s, 1/s, -MAGIC/s]
                sq = const_pool.tile([1, 3], F32, tag="sq")
                rinv = const_pool.tile([1, 1], F32, tag="rinv")
                newton_recip(rinv, rng, "rr")
                nc.vector.tensor_scalar_mul(sq[:, 0:1], rinv, 255.0)
                newton_recip(sq[:, 1:2], sq[:, 0:1], "si")
                nc.vector.tensor_scalar_mul(sq[:, 2:3], sq[:, 1:2], -MAGIC)
                sq_b = const_pool.tile([128, 3], F32, tag="sq_b")
                nc.gpsimd.partition_broadcast(sq_b, sq)

                # ---- requant tail: out = (rne(y*s) - MAGIC)/s, f16 ----
                N_DVE = 21  # tiles handled by DVE; rest on ScalarE
                for i in range(NB):
                    q = wpool.tile([128, OUT_PC], F32, tag="wslab", bufs=3)
                    o = wpool.tile([128, OUT_PC], F16, tag="b16", bufs=2)
                    if i < N_DVE:
                        nc.vector.tensor_scalar(
                            q, y[:, i, :], sq_b[:, 0:1], MAGIC,
                            mybir.AluOpType.mult, mybir.AluOpType.add,
                        )
                        nc.vector.tensor_scalar(
                            o, q, MAGIC, sq_b[:, 1:2],
                            mybir.AluOpType.subtract, mybir.AluOpType.mult,
                        )
                    else:
                        nc.scalar.activation(
                            q, y[:, i, :], mybir.ActivationFunctionType.Copy,
                            bias=MAGIC, scale=sq_b[:, 0:1],
                        )
                        nc.scalar.activation(
                            o, q, mybir.ActivationFunctionType.Identity,
                            bias=sq_b[:, 2:3], scale=sq_b[:, 1:2],
                        )
                    nc.sync.dma_start(out_ap[i * TOKB:(i + 1) * TOKB, :], o)

            for _ in range(repeat):
                body()

    nc.compile()
    return nc


_NC_CACHE: dict = {}


def _get_full_nc():
    if "full" not in _NC_CACHE:
        _NC_CACHE["full"] = build_kernel()
    return _NC_CACHE["full"]


def make_in_maps(x, weight, bias, scale):
    """Host-side layout prep: transpose/cast/shard. No arithmetic on values."""
    x = np.asarray(x, dtype=np.float32)
    weight = np.asarray(weight, dtype=np.float32)
    bias = np.asarray(bias, dtype=np.float32)
    scale = np.asarray(scale, dtype=np.float32)

    n_tok = x.size // x.shape[-1]
    k_dim = x.shape[-1]
    d_out = weight.shape[0]

    xf = x.reshape(n_tok, k_dim).astype(np.float16)
    wt_full = np.ascontiguousarray(weight.T)  # [k, d_out]

    in_maps = []
    for cid in range(GRID_R * GRID_C):
        r, c = divmod(cid, GRID_C)
        xs = xf[r * TOK_PC:(r + 1) * TOK_PC]          # [tok_pc, k]
        # [NB, TOKB, KO, 128] -> [NB, 128, KO, TOKB]
        xs = np.ascontiguousarray(
            xs.reshape(NB, TOKB, KO, 128).transpose(0, 3, 2, 1)
        )
        wt_c = np.ascontiguousarray(wt_full[:, c * OUT_PC:(c + 1) * OUT_PC])
        wt1_c = np.ascontiguousarray(
            wt_c[r * (k_dim // 2):(r + 1) * (k_dim // 2), :]
        )
        in_maps.append(
            {
                "xt": xs,
                "wt": wt_c,
                "wt1": wt1_c,
                "biasv": np.ascontiguousarray(bias[c * OUT_PC:(c + 1) * OUT_PC].astype(np.float16)),
                "scalev": scale.reshape(1),
            }
        )
    return in_maps


def assemble_out(results, out_shape):
    n_tok = int(np.prod(out_shape[:-1]))
    d_out = out_shape[-1]
    full = np.empty((n_tok, d_out), dtype=np.float32)
    for cid in range(GRID_R * GRID_C):
        r, c = divmod(cid, GRID_C)
        full[r * TOK_PC:(r + 1) * TOK_PC, c * OUT_PC:(c + 1) * OUT_PC] = (
            results[cid]["outv"].astype(np.float32)
        )
    return full.reshape(out_shape)


def kernel(x, weight, bias, scale):
    nc = _get_full_nc()
    in_maps = make_in_maps(x, weight, bias, scale)
    res = bass_utils.run_bass_kernel_spmd(nc, in_maps, core_ids=list(range(N_CORES)))
    return assemble_out(res.results, (B, S, D_OUT))


# revision 12
# speedup vs baseline: 1.3073x; 1.3073x over previous
"""BitLinear (ternary-weight linear + global activation requant) on 8 TRN2 cores.

Computation (see reference):
    wq  = ternarize(weight * scale, thr = 0.7*mean|weight*scale|)   # {-1,0,+1}
    out = x @ wq.T + bias
    s   = 255 / (max(out) - min(out));  out = round(out*s)/s

Sharding: 2x4 grid over (tokens, out_features).  Each core computes a
[4096 tok, 1024 out] shard contracting over the full K=4096.

v2 design (vs the staged baseline):
  - fp16 x / fp16 wq matmuls (same PE rate as bf16, much lower quant error)
  - pass-1 |W| scan split across the 2 row-replica cores (half the DMA)
  - ternarize via clamp(rne(w*beta), -1, 1) in 3 cheap DVE/ScalarE ops,
    streamed under the start of the MM phase (slice-level wq deps)
  - raw y kept resident in SBUF as f16 -- no DRAM staging round trip
  - first 4 blocks interleaved ko-major so the PE rides the pass-2 W DMA
    stream; remaining blocks sequential with pipelined PSUM banks
  - tiny collectives as AllGather (4.6us floor) + local reduce
  - requant tail split across DVE and ScalarE, f16 output (host upcasts)
"""

import numpy as np
import ml_dtypes

import concourse.bass as bass
import concourse.mybir as mybir
import concourse.tile as tile
from concourse.tile import add_dep_helper
from concourse import bacc
from concourse import bass_utils

F32 = mybir.dt.float32
F16 = mybir.dt.float16

# Full problem shape
B, S, D_IN, D_OUT = 4, 2048, 4096, 4096
N_CORES = 8
GRID_R, GRID_C = 2, 4  # token shards x out-feature shards

TOK_PC = (B * S) // GRID_R      # 4096 tokens per core
OUT_PC = D_OUT // GRID_C        # 1024 out features per core
KO = D_IN // 128                # 32 contraction slabs
TOKB = 128                      # tokens per block
NB = TOK_PC // TOKB             # 32 blocks
OGS, OGW = 2, 512               # out groups per block (PSUM tiles)
NFLIGHT = 4                     # blocks in flight (NFLIGHT*OGS = 8 PSUM banks)

MAGIC32 = 12582912.0            # f32 RNE magic (1.5 * 2^23)
MAGIC16 = 1536.0                # f16 RNE magic (1.5 * 2^10)


def build_kernel(debug: bool = False, repeat: int = 1,
                 magic16: bool = False, newton1: bool = False,
                 tail16: bool = False, wide: bool = False):
    nc = bacc.Bacc(
        "TRN2",
        target_bir_lowering=False,
        debug=debug,
        enable_asserts=False,
        num_devices=N_CORES,
    )

    xt = nc.declare_dram_parameter("xt", [NB, 128, KO, TOKB], F16, isOutput=False)
    wt = nc.declare_dram_parameter("wt", [D_IN, OUT_PC], F32, isOutput=False)
    wt1 = nc.declare_dram_parameter("wt1", [D_IN // 2, OUT_PC], F32, isOutput=False)
    biasv = nc.declare_dram_parameter("biasv", [OUT_PC], F16, isOutput=False)
    scalev = nc.declare_dram_parameter("scalev", [1], F32, isOutput=False)
    out = nc.declare_dram_parameter("outv", [TOK_PC, OUT_PC], F16, isOutput=True)

    xt_ap = xt.ap()
    wt_ap = wt.ap()
    wt1_ap = wt1.ap()
    out_ap = out.ap()

    with tile.TileContext(nc) as tc:
        with (
            tc.tile_pool(name="const", bufs=1) as const_pool,
            tc.tile_pool(name="wpool", bufs=2) as wpool,
            tc.tile_pool(name="wq", bufs=1) as wq_pool,
            tc.tile_pool(name="xbuf", bufs=5) as x_pool,
            tc.tile_pool(name="ybuf", bufs=1) as y_pool,
            tc.tile_pool(name="psum", bufs=1, space="PSUM") as psum_pool,
            tc.tile_pool(name="dram", bufs=1, space="DRAM") as dram_pool,
        ):

            def newton_recip(out_ap2, in_ap, tag):
                # r_{i+1} = r_i*(2 - x*r_i), two Newton steps on InstReciprocal
                r0 = const_pool.tile([1, 1], F32, tag=f"{tag}_r0")
                nc.vector.reciprocal(r0, in_ap)
                e = const_pool.tile([1, 1], F32, tag=f"{tag}_e")
                for _ in range(1 if newton1 else 2):
                    nc.vector.tensor_scalar(
                        e, in_ap, r0, None, mybir.AluOpType.mult
                    )
                    nc.vector.tensor_scalar(
                        e, e, -1.0, 2.0, mybir.AluOpType.mult, mybir.AluOpType.add
                    )
                    nc.vector.tensor_mul(r0, r0, e)
                nc.vector.tensor_copy(out=out_ap2, in_=r0)

            def body():
                # ---- consts ----
                scale_sb = const_pool.tile([1, 1], F32, tag="scale_sb")
                nc.sync.dma_start(scale_sb, scalev.ap()[None, :])
                bias_sb = const_pool.tile([1, OUT_PC], F16, tag="bias_sb")
                nc.sync.dma_start(bias_sb, biasv.ap()[None, :])
                bias_b = const_pool.tile([128, OUT_PC], F16, tag="bias_b")
                nc.gpsimd.partition_broadcast(bias_b, bias_sb)

                # ---- pass-1: |W| sum over this core's half of its W shard ----
                wsum = const_pool.tile([128, KO // 2], F32, tag="wsum")
                p1_last = None
                for k in range(KO // 2):
                    s1 = wpool.tile([128, OUT_PC], F32, tag="wslab", bufs=3)
                    p1_last = nc.sync.dma_start(
                        s1, wt1_ap[k * 128:(k + 1) * 128, :]
                    )
                    nc.vector.tensor_reduce(
                        wsum[:, k:k + 1], s1,
                        axis=mybir.AxisListType.X,
                        op=mybir.AluOpType.add, apply_absolute_value=True,
                    )
                wsum1 = const_pool.tile([128, 1], F32, tag="wsum1")
                nc.vector.tensor_reduce(
                    wsum1, wsum, axis=mybir.AxisListType.X, op=mybir.AluOpType.add
                )
                wsum_all = const_pool.tile([128, 1], F32, tag="wsum_all")
                nc.gpsimd.partition_all_reduce(
                    wsum_all, wsum1, 128, bass.bass_isa.ReduceOp.add
                )
                cc1_in = dram_pool.tile([1, 1], F32, tag="cc1_in")
                cc1_out = dram_pool.tile([N_CORES, 1], F32, tag="cc1_out")
                nc.sync.dma_start(cc1_in, wsum_all[0:1, :])
                nc.gpsimd.collective_compute(
                    "AllGather",
                    mybir.AluOpType.bypass,
                    replica_groups=[list(range(N_CORES))],
                    ins=[cc1_in.opt()],
                    outs=[cc1_out.opt()],
                )
                s8 = const_pool.tile([1, N_CORES], F32, tag="s8")
                nc.sync.dma_start(s8, cc1_out.rearrange("g c -> c g"))
                s_glob = const_pool.tile([1, 1], F32, tag="s_glob")
                nc.vector.tensor_reduce(
                    s_glob, s8, axis=mybir.AxisListType.X, op=mybir.AluOpType.add
                )

                # ---- beta = 0.5*scale / thr;  thr = 0.7/n * S_glob * |scale| ----
                absscale = const_pool.tile([1, 1], F32, tag="absscale")
                nc.vector.tensor_reduce(
                    absscale, scale_sb, axis=mybir.AxisListType.X,
                    op=mybir.AluOpType.max, apply_absolute_value=True,
                )
                r1 = const_pool.tile([1, 1], F32, tag="r1")
                newton_recip(r1, s_glob, "r1")
                r2 = const_pool.tile([1, 1], F32, tag="r2")
                newton_recip(r2, absscale, "r2")
                n_w = float(D_IN) * float(D_OUT)
                bco = float(np.float32(0.5 * n_w / 0.7))
                beta = const_pool.tile([1, 1], F32, tag="beta")
                nc.vector.tensor_mul(beta, r1, r2)
                nc.vector.tensor_mul(beta, beta, scale_sb)
                nc.vector.tensor_scalar_mul(beta, beta, bco)
                beta_b = const_pool.tile([128, 1], F32, tag="beta_b")
                nc.gpsimd.partition_broadcast(beta_b, beta)

                # ---- x prefetch for the first NFLIGHT blocks ----
                xts = {}
                prev = p1_last
                for b in range(NFLIGHT):
                    xtile = x_pool.tile([128, KO, TOKB], F16, tag="x_tile")
                    d = nc.sync.dma_start(xtile, xt_ap[b])
                    add_dep_helper(d.ins, prev.ins, sync=True,
                                   reason="x prefetch after pass-1 / chain")
                    prev = d
                    xts[b] = xtile

                # ---- pass-2 W stream + ternarize into wq (f16) ----
                wq = wq_pool.tile([128, KO, OUT_PC], F16, tag="wq")
                d2first = None
                for ko in range(KO):
                    s2 = wpool.tile([128, OUT_PC], F32, tag="wslab", bufs=3)
                    d2 = nc.sync.dma_start(s2, wt_ap[ko * 128:(ko + 1) * 128, :])
                    if d2first is None:
                        add_dep_helper(d2.ins, p1_last.ins, sync=True,
                                       reason="pass-2 W after pass-1 W DMA")
                        d2first = d2
                    if magic16:
                        a = wpool.tile([128, OUT_PC], F16, tag="a16", bufs=2)
                        cmag = MAGIC16
                    else:
                        a = wpool.tile([128, OUT_PC], F32, tag="a32", bufs=3)
                        cmag = MAGIC32
                    nc.vector.tensor_scalar(
                        a, s2, beta_b[:, 0:1], cmag,
                        mybir.AluOpType.mult, mybir.AluOpType.add,
                    )
                    bt = wpool.tile([128, OUT_PC], F16, tag="b16", bufs=2)
                    nc.scalar.activation(
                        bt, a, mybir.ActivationFunctionType.Copy,
                        bias=-cmag, scale=1.0,
                    )
                    nc.vector.tensor_scalar(
                        wq[:, ko, :], bt, 1.0, -1.0,
                        mybir.AluOpType.min, mybir.AluOpType.max,
                    )

                # ---- MM phase ----
                y = y_pool.tile([128, NB, OUT_PC], F16, tag="y")
                maxst = const_pool.tile([128, NB * OGS], F32, tag="maxst")
                minst = const_pool.tile([128, NB * OGS], F32, tag="minst")

                ogs, ogw = (1, OUT_PC) if wide else (OGS, OGW)

                def psums_for(b):
                    return [
                        psum_pool.tile([128, ogw], F32, name=f"ps_{b % NFLIGHT}_{og}")
                        for og in range(ogs)
                    ]

                def drain(b, ps):
                    for og in range(ogs):
                        sl = slice(og * ogw, (og + 1) * ogw)
                        nc.vector.tensor_add(y[:, b, sl], ps[og], bias_b[:, sl])
                        idx = b * ogs + og
                        nc.vector.tensor_reduce(
                            maxst[:, idx:idx + 1], y[:, b, sl],
                            axis=mybir.AxisListType.X, op=mybir.AluOpType.max,
                        )
                        nc.vector.tensor_reduce(
                            minst[:, idx:idx + 1], y[:, b, sl],
                            axis=mybir.AxisListType.X, op=mybir.AluOpType.min,
                        )

                # phase A: first NFLIGHT blocks interleaved ko-major, pacing
                # the PE to the pass-2 DMA/ternarize stream
                psA = {b: psums_for(b) for b in range(NFLIGHT)}
                for ko in range(KO):
                    for b in range(NFLIGHT):
                        lhsT = xts[b][:, ko, :]
                        for og in range(ogs):
                            nc.tensor.matmul(
                                psA[b][og], lhsT,
                                wq[:, ko, og * ogw:(og + 1) * ogw],
                                start=(ko == 0), stop=(ko == KO - 1),
                            )
                # prefetch the next two blocks' x while phase A computes
                for pb in (NFLIGHT, NFLIGHT + 1):
                    if pb < NB:
                        xp = x_pool.tile([128, KO, TOKB], F16, tag="x_tile")
                        nc.sync.dma_start(xp, xt_ap[pb])
                        xts[pb] = xp
                for b in range(NFLIGHT):
                    drain(b, psA[b])

                # phase B: remaining blocks, sequential with pipelined banks
                for b in range(NFLIGHT, NB):
                    # prefetch x two blocks ahead
                    pf = b + 2 if b + 2 < NB else None
                    xtile = xts.pop(b, None)
                    if xtile is None:
                        xtile = x_pool.tile([128, KO, TOKB], F16, tag="x_tile")
                        nc.sync.dma_start(xtile, xt_ap[b])
                    if pf is not None and pf not in xts:
                        xp = x_pool.tile([128, KO, TOKB], F16, tag="x_tile")
                        nc.sync.dma_start(xp, xt_ap[pf])
                        xts[pf] = xp
                    ps = psums_for(b)
                    for ko in range(KO):
                        lhsT = xtile[:, ko, :]
                        for og in range(ogs):
                            nc.tensor.matmul(
                                ps[og], lhsT,
                                wq[:, ko, og * ogw:(og + 1) * ogw],
                                start=(ko == 0), stop=(ko == KO - 1),
                            )
                    drain(b, ps)

                # ---- global max/min -> s ----
                lmax = const_pool.tile([128, 1], F32, tag="lmax")
                lmin = const_pool.tile([128, 1], F32, tag="lmin")
                nc.vector.tensor_reduce(
                    lmax, maxst, axis=mybir.AxisListType.X, op=mybir.AluOpType.max
                )
                nc.vector.tensor_reduce(
                    lmin, minst, axis=mybir.AxisListType.X, op=mybir.AluOpType.min
                )
                st2 = const_pool.tile([128, 2], F32, tag="st2")
                nc.vector.tensor_copy(out=st2[:, 0:1], in_=lmax)
                nc.vector.tensor_scalar_mul(st2[:, 1:2], lmin, -1.0)
                st2r = const_pool.tile([128, 2], F32, tag="st2r")
                nc.gpsimd.partition_all_reduce(
                    st2r, st2, 128, bass.bass_isa.ReduceOp.max
                )
                cc2_in = dram_pool.tile([1, 2], F32, tag="cc2_in")
                cc2_out = dram_pool.tile([N_CORES, 2], F32, tag="cc2_out")
                nc.sync.dma_start(cc2_in, st2r[0:1, :])
                nc.gpsimd.collective_compute(
                    "AllGather",
                    mybir.AluOpType.bypass,
                    replica_groups=[list(range(N_CORES))],
                    ins=[cc2_in.opt()],
                    outs=[cc2_out.opt()],
                )
                gst = const_pool.tile([1, 2, N_CORES], F32, tag="gst")
                nc.sync.dma_start(gst, cc2_out.rearrange("g c -> c g")[None])
                g2 = const_pool.tile([1, 2, 1], F32, tag="g2")
                nc.vector.tensor_reduce(
                    g2, gst, axis=mybir.AxisListType.X, op=mybir.AluOpType.max
                )
                rng = const_pool.tile([1, 1], F32, tag="rng")  # max - min
                nc.vector.tensor_reduce(
                    rng, g2[:, :, 0], axis=mybir.AxisListType.X,
                    op=mybir.AluOpType.add,
                )
                # sq = [s, 1/s, -MAGIC/s]
                sq = const_pool.tile([1, 3], F32, tag="sq")
                rinv = const_pool.tile([1, 1], F32, tag="rinv")
                newton_recip(rinv, rng, "rr")
                nc.vector.tensor_scalar_mul(sq[:, 0:1], rinv, 255.0)
                newton_recip(sq[:, 1:2], sq[:, 0:1], "si")
                nc.vector.tensor_scalar_mul(sq[:, 2:3], sq[:, 1:2], -(MAGIC16 if tail16 else MAGIC32))
                sq_b = const_pool.tile([128, 3], F32, tag="sq_b")
                nc.gpsimd.partition_broadcast(sq_b, sq)

                # ---- requant tail: out = (rne(y*s) - MAGIC)/s, f16 ----
                tmag = MAGIC16 if tail16 else MAGIC32
                for i in range(NB):
                    if tail16:
                        q = wpool.tile([128, OUT_PC], F16, tag="a16", bufs=2)
                    else:
                        q = wpool.tile([128, OUT_PC], F32, tag="a32", bufs=3)
                    o = wpool.tile([128, OUT_PC], F16, tag="b16", bufs=2)
                    if i % 3 != 2:
                        nc.vector.tensor_scalar(
                            q, y[:, i, :], sq_b[:, 0:1], tmag,
                            mybir.AluOpType.mult, mybir.AluOpType.add,
                        )
                        nc.vector.tensor_scalar(
                            o, q, tmag, sq_b[:, 1:2],
                            mybir.AluOpType.subtract, mybir.AluOpType.mult,
                        )
                    else:
                        nc.scalar.activation(
                            q, y[:, i, :], mybir.ActivationFunctionType.Copy,
                            bias=tmag, scale=sq_b[:, 0:1],
                        )
                        nc.scalar.activation(
                            o, q, mybir.ActivationFunctionType.Identity,
                            bias=sq_b[:, 2:3], scale=sq_b[:, 1:2],
                        )
                    nc.sync.dma_start(out_ap[i * TOKB:(i + 1) * TOKB, :], o)

            for _ in range(repeat):
                body()

    nc.compile()
    return nc


_NC_CACHE: dict = {}


def _get_full_nc():
    if "full" not in _NC_CACHE:
        _NC_CACHE["full"] = build_kernel()
    return _NC_CACHE["full"]


def make_in_maps(x, weight, bias, scale):
    """Host-side layout prep: transpose/cast/shard. No arithmetic on values."""
    x = np.asarray(x, dtype=np.float32)
    weight = np.asarray(weight, dtype=np.float32)
    bias = np.asarray(bias, dtype=np.float32)
    scale = np.asarray(scale, dtype=np.float32)

    n_tok = x.size // x.shape[-1]
    k_dim = x.shape[-1]
    d_out = weight.shape[0]

    xf = x.reshape(n_tok, k_dim).astype(np.float16)
    wt_full = np.ascontiguousarray(weight.T)  # [k, d_out]

    in_maps = []
    for cid in range(GRID_R * GRID_C):
        r, c = divmod(cid, GRID_C)
        xs = xf[r * TOK_PC:(r + 1) * TOK_PC]          # [tok_pc, k]
        # [NB, TOKB, KO, 128] -> [NB, 128, KO, TOKB]
        xs = np.ascontiguousarray(
            xs.reshape(NB, TOKB, KO, 128).transpose(0, 3, 2, 1)
        )
        wt_c = np.ascontiguousarray(wt_full[:, c * OUT_PC:(c + 1) * OUT_PC])
        wt1_c = np.ascontiguousarray(
            wt_c[r * (k_dim // 2):(r + 1) * (k_dim // 2), :]
        )
        in_maps.append(
            {
                "xt": xs,
                "wt": wt_c,
                "wt1": wt1_c,
                "biasv": np.ascontiguousarray(bias[c * OUT_PC:(c + 1) * OUT_PC].astype(np.float16)),
                "scalev": scale.reshape(1),
            }
        )
    return in_maps


def assemble_out(results, out_shape):
    n_tok = int(np.prod(out_shape[:-1]))
    d_out = out_shape[-1]
    full = np.empty((n_tok, d_out), dtype=np.float32)
    for cid in range(GRID_R * GRID_C):
        r, c = divmod(cid, GRID_C)
        full[r * TOK_PC:(r + 1) * TOK_PC, c * OUT_PC:(c + 1) * OUT_PC] = (
            results[cid]["outv"].astype(np.float32)
        )
    return full.reshape(out_shape)


def kernel(x, weight, bias, scale):
    nc = _get_full_nc()
    in_maps = make_in_maps(x, weight, bias, scale)
    res = bass_utils.run_bass_kernel_spmd(nc, in_maps, core_ids=list(range(N_CORES)))
    return assemble_out(res.results, (B, S, D_OUT))
